# revision 1
# baseline (speedup 1.0000x reference)
"""Adaptive Spectral Block on 8 TRN2 NeuronCores (data-parallel over batch).

N = 4097 = 241*17 Cooley-Tukey factored FFT as matmuls:
  fwd (fp32):  stage1 A1 [K=n1(241), M=(re 0:121|pad|im 128:248)] -> t tiles,
               corner-turn DMA -> per-triple moving tiles [102, 256],
               stage2 A2_g [102, 116] -> X bins in class layout
               [re (i,k2) 0:51 | pad | im 64:115].
  spectral:    energy via ACT square-accum from PSUM (f32), exact median via
               31-iter radix select on float bits, mask, complex multiply
               (bf16) with conj-sign for k>2048 representatives.
  inv (bf16):  stageA Ainv_g [116,116] -> s tiles, corner-turn2 -> [241, n2*c],
               stageB B1 -> out rows 17*n1+n2.
"""
import numpy as np
import ml_dtypes

B, N, C = 32, 4097, 256
F = N // 2 + 1
BL = B // 8
NSQ = np.sqrt(np.float64(N))
FW = 17 * C  # 4352
NCH = 9      # 8x512 + 256 free chunks


def _build_consts():
    n1 = np.arange(241)
    k1 = np.arange(121)
    n2 = np.arange(17)
    k2 = np.arange(17)

    ang = 2 * np.pi * np.outer(n1, k1) / 241.0
    A1 = np.zeros((241, 256), np.float64)
    A1[:, 0:121] = np.cos(ang)
    A1[:, 128:248] = -np.sin(ang[:, 1:121])
    A1 /= NSQ

    def cls_mat(c):
        kk = c + 241 * k2
        th = -2 * np.pi * np.outer(n2, kk) / N
        Cm, Sm = np.cos(th), np.sin(th)
        M = np.zeros((34, 34))
        M[0:17, 0:17] = Cm
        M[17:34, 0:17] = -Sm
        M[0:17, 17:34] = Sm
        M[17:34, 17:34] = Cm
        return M

    A2_all = np.zeros((102, 40, 116), np.float64)
    for g in range(40):
        for i in range(3):
            c = 3 * g + 1 + i
            M = cls_mat(c)
            A2_all[17 * i:17 * i + 17, g, 17 * i:17 * i + 17] = M[0:17, 0:17]
            A2_all[51 + 17 * i:51 + 17 * i + 17, g, 17 * i:17 * i + 17] = M[17:34, 0:17]
            A2_all[17 * i:17 * i + 17, g, 64 + 17 * i:64 + 17 * i + 17] = M[0:17, 17:34]
            A2_all[51 + 17 * i:51 + 17 * i + 17, g, 64 + 17 * i:64 + 17 * i + 17] = M[17:34, 17:34]
    A2_all = A2_all.reshape(102, 40 * 116)

    kk0 = 241 * np.arange(9)
    th0 = -2 * np.pi * np.outer(n2, kk0) / N
    A2_0 = np.zeros((17, 42), np.float64)
    A2_0[:, 0:9] = np.cos(th0)
    A2_0[:, 32:41] = np.sin(th0)

    def cls_inv(c):
        kk = c + 241 * k2
        th = +2 * np.pi * np.outer(n2, kk) / N
        Cm, Sm = np.cos(th), np.sin(th)
        M = np.zeros((34, 34))
        M[0:17, 0:17] = Cm.T
        M[17:34, 0:17] = -Sm.T
        M[0:17, 17:34] = Sm.T
        M[17:34, 17:34] = Cm.T
        return M / NSQ

    Ainv_all = np.zeros((116, 40, 116), np.float64)
    for g in range(40):
        for i in range(3):
            c = 3 * g + 1 + i
            M = cls_inv(c)
            Ainv_all[17 * i:17 * i + 17, g, 17 * i:17 * i + 17] = M[0:17, 0:17]
            Ainv_all[64 + 17 * i:64 + 17 * i + 17, g, 17 * i:17 * i + 17] = M[17:34, 0:17]
            Ainv_all[17 * i:17 * i + 17, g, 64 + 17 * i:64 + 17 * i + 17] = M[0:17, 17:34]
            Ainv_all[64 + 17 * i:64 + 17 * i + 17, g, 64 + 17 * i:64 + 17 * i + 17] = M[17:34, 17:34]
    Ainv_all = Ainv_all.reshape(116, 40 * 116)

    th = 2 * np.pi * np.outer(np.arange(9), n2) / 17.0
    Ainv0 = np.zeros((42, 18), np.float64)
    Ainv0[0, 0:17] = 1.0
    Ainv0[1:9, 0:17] = 2 * np.cos(th[1:9])
    Ainv0[33:41, 0:17] = -2 * np.sin(th[1:9])
    Ainv0 /= NSQ

    ang2 = 2 * np.pi * np.outer(k1, n1) / 241.0
    ck = np.where(k1 == 0, 1.0, 2.0)
    cosr = ck[:, None] * np.cos(ang2)
    sinr = -2.0 * np.sin(ang2[1:121])
    B1 = np.zeros((241, 256), np.float64)
    B1[0:121, 0:128] = cosr[:, 0:128]
    B1[0:121, 128:241] = cosr[:, 128:241]
    B1[121:128, 0:128] = sinr[0:7, 0:128]
    B1[121:128, 128:241] = sinr[0:7, 128:241]
    B1[128:241, 0:128] = sinr[7:120, 0:128]
    B1[128:241, 128:241] = sinr[7:120, 128:241]

    binm = np.zeros((42, 51), np.int64)
    sgn = np.ones((40, 51), np.float64)
    for g in range(40):
        rg = g + (1 if g >= 32 else 0)
        for i in range(3):
            c = 3 * g + 1 + i
            for q in range(17):
                k = c + 241 * q
                binm[rg, 17 * i + q] = k if k <= 2048 else N - k
                if k > 2048:
                    sgn[g, 17 * i + q] = -1.0
    for q in range(9):
        binm[32, q] = 241 * q
    SGN2 = np.zeros((116, 42), np.float64)
    for g in range(40):
        rg = g + (1 if g >= 32 else 0)
        SGN2[0:51, rg] = sgn[g]
        SGN2[64:115, rg] = sgn[g]
    SGN2[:, 32] = 1.0

    bitsr = np.zeros((1, 124), np.uint32)
    for b in range(31):
        bitsr[0, 4 * b:4 * b + 4] = np.uint32(1 << b)
    ones128 = np.ones((1, 128), np.float32)
    ones41 = np.zeros((42, 2), np.float32)
    ones41[:, 0] = 1.0

    bf = ml_dtypes.bfloat16
    return {
        "A1": A1.astype(np.float32), "A2": A2_all.astype(np.float32),
        "A20": A2_0.astype(np.float32), "AINV": Ainv_all.astype(bf),
        "AINV0": Ainv0.astype(bf), "B1": B1.astype(bf),
        "SGN2": SGN2.astype(bf), "BITS": bitsr,
        "ONES128": ones128, "ONES41": ones41,
    }, binm


_CONSTS, _BINM = _build_consts()
_NC_CACHE = {}


def _build_nc():
    if "nc" in _NC_CACHE:
        return _NC_CACHE["nc"]
    from contextlib import ExitStack
    from concourse import bacc, tile, mybir
    f32 = mybir.dt.float32
    bf16 = mybir.dt.bfloat16
    u32 = mybir.dt.uint32
    Alu = mybir.AluOpType
    Act = mybir.ActivationFunctionType

    nc = bacc.Bacc("TRN2", target_bir_lowering=False, debug=False, num_devices=8)
    x_t = nc.dram_tensor("x", [BL, N, C], f32, kind="ExternalInput")
    thr_t = nc.dram_tensor("thrp", [42, 51], f32, kind="ExternalInput")
    wre_t = nc.dram_tensor("wre", [116, C], bf16, kind="ExternalInput")
    wim_t = nc.dram_tensor("wim", [116, C], bf16, kind="ExternalInput")
    whre_t = nc.dram_tensor("whre", [116, C], bf16, kind="ExternalInput")
    whim_t = nc.dram_tensor("whim", [116, C], bf16, kind="ExternalInput")
    a1_t = nc.dram_tensor("A1", [241, 256], f32, kind="ExternalInput")
    a2_t = nc.dram_tensor("A2", [102, 40 * 116], f32, kind="ExternalInput")
    a20_t = nc.dram_tensor("A20", [17, 42], f32, kind="ExternalInput")
    ainv_t = nc.dram_tensor("AINV", [116, 40 * 116], bf16, kind="ExternalInput")
    ainv0_t = nc.dram_tensor("AINV0", [42, 18], bf16, kind="ExternalInput")
    b1_t = nc.dram_tensor("B1", [241, 256], bf16, kind="ExternalInput")
    sgn2_t = nc.dram_tensor("SGN2", [116, 42], bf16, kind="ExternalInput")
    bits_t = nc.dram_tensor("BITS", [1, 124], u32, kind="ExternalInput")
    o128_t = nc.dram_tensor("ONES128", [1, 128], f32, kind="ExternalInput")
    o41_t = nc.dram_tensor("ONES41", [42, 2], f32, kind="ExternalInput")
    out_t = nc.dram_tensor("out", [BL, N, C], f32, kind="ExternalOutput")
    dbgm_t = nc.dram_tensor("dbg_med", [1, 4], f32, kind="ExternalOutput")
    dbge_t = nc.dram_tensor("dbg_e", [42, 51], f32, kind="ExternalOutput")

    def chunk(fc):
        lo = fc * 512
        return lo, min(512, FW - lo)

    with tile.TileContext(nc) as tc, ExitStack() as ES:
        cpool = ES.enter_context(tc.tile_pool(name="consts", bufs=1))
        x_p = ES.enter_context(tc.tile_pool(name="xcls", bufs=1))
        e_p = ES.enter_context(tc.tile_pool(name="energy", bufs=2))
        med_p = ES.enter_context(tc.tile_pool(name="med", bufs=1))

        a1k0 = cpool.tile([128, 256], f32)
        a1k1 = cpool.tile([113, 256], f32)
        nc.sync.dma_start(out=a1k0, in_=a1_t.ap()[0:128, :])
        nc.sync.dma_start(out=a1k1, in_=a1_t.ap()[128:241, :])
        a2_sb = cpool.tile([102, 40 * 116], f32)
        nc.sync.dma_start(out=a2_sb, in_=a2_t.ap())
        a20_sb = cpool.tile([17, 42], f32)
        nc.sync.dma_start(out=a20_sb, in_=a20_t.ap())
        ainv_sb = cpool.tile([116, 40 * 116], bf16)
        nc.sync.dma_start(out=ainv_sb, in_=ainv_t.ap())
        ainv0_sb = cpool.tile([42, 18], bf16)
        nc.sync.dma_start(out=ainv0_sb, in_=ainv0_t.ap())
        b1k0 = cpool.tile([128, 256], bf16)
        b1k1 = cpool.tile([113, 256], bf16)
        nc.sync.dma_start(out=b1k0, in_=b1_t.ap()[0:128, :])
        nc.sync.dma_start(out=b1k1, in_=b1_t.ap()[128:241, :])
        wre = cpool.tile([116, C], bf16)
        wim = cpool.tile([116, C], bf16)
        whre = cpool.tile([116, C], bf16)
        whim = cpool.tile([116, C], bf16)
        nc.sync.dma_start(out=wre, in_=wre_t.ap())
        nc.sync.dma_start(out=wim, in_=wim_t.ap())
        nc.sync.dma_start(out=whre, in_=whre_t.ap())
        nc.sync.dma_start(out=whim, in_=whim_t.ap())
        sgn2 = cpool.tile([116, 42], bf16)
        nc.sync.dma_start(out=sgn2, in_=sgn2_t.ap())
        bits = cpool.tile([1, 124], u32)
        nc.sync.dma_start(out=bits, in_=bits_t.ap())
        o128 = cpool.tile([1, 128], f32)
        nc.sync.dma_start(out=o128, in_=o128_t.ap())
        o41 = cpool.tile([42, 2], f32)
        nc.sync.dma_start(out=o41, in_=o41_t.ap())
        thrp = cpool.tile([42, 51], f32)
        nc.sync.dma_start(out=thrp, in_=thr_t.ap())

        all_ebins = []
        all_x = []

        # ================= phase 1: forward =================
        with tc.tile_pool(name="xin", bufs=1) as xin_p, \
             tc.tile_pool(name="s1ps", bufs=2, space="PSUM") as s1ps_p, \
             tc.tile_pool(name="tsb", bufs=1) as t_p, \
             tc.tile_pool(name="mt", bufs=6) as m_p, \
             tc.tile_pool(name="s2ps", bufs=4, space="PSUM") as s2ps_p:
            for s in range(BL):
                xv = x_t.ap().rearrange("s (a b) c -> s a b c", a=241, b=17)
                xin0 = xin_p.tile([128, FW], f32, tag="xin0")
                xin1 = xin_p.tile([113, FW], f32, tag="xin1")
                nc.sync.dma_start(out=xin0, in_=xv[s:s + 1, 0:128])
                nc.sync.dma_start(out=xin1, in_=xv[s:s + 1, 128:241])
                t0 = t_p.tile([128, FW], f32, tag="t0")
                t1 = t_p.tile([120, FW], f32, tag="t1")
                for mt in range(2):
                    for fc in range(NCH):
                        lo, w = chunk(fc)
                        ps = s1ps_p.tile([128, 512], f32, tag="s1")
                        nc.tensor.matmul(ps[:, 0:w],
                                         a1k0[:, 128 * mt:128 * mt + 128],
                                         xin0[:, lo:lo + w],
                                         start=True, stop=False)
                        nc.tensor.matmul(ps[:, 0:w],
                                         a1k1[:, 128 * mt:128 * mt + 128],
                                         xin1[:, lo:lo + w],
                                         start=False, stop=True)
                        dst, rows = (t0, 128) if mt == 0 else (t1, 120)
                        nc.vector.tensor_copy(out=dst[0:rows, lo:lo + w],
                                              in_=ps[0:rows, 0:w])

                e2 = e_p.tile([128, 64], f32, tag="e2")
                nc.vector.memset(e2, 0.0)
                nc.vector.memset(e2[0:128, 41:42], 5.0e29)
                xts = []
                for g in range(40):
                    mt_g = m_p.tile([102, 256], f32, tag="m")
                    nc.scalar.dma_start(
                        out=mt_g[0:51, :],
                        in_=t0[3 * g + 1:3 * g + 4, :].rearrange(
                            "i (q c) -> i q c", q=17, c=256))
                    nc.gpsimd.dma_start(
                        out=mt_g[51:102, :],
                        in_=t1[3 * g:3 * g + 3, :].rearrange(
                            "i (q c) -> i q c", q=17, c=256))
                    xps = s2ps_p.tile([116, 256], f32, tag="x")
                    nc.tensor.matmul(xps, a2_sb[:, 116 * g:116 * g + 116],
                                     mt_g, start=True, stop=True)
                    rg = g + (1 if g >= 32 else 0)
                    ej = e_p.tile([116, 256], f32, tag="ejunk")
                    nc.scalar.activation(out=ej, in_=xps, func=Act.Square,
                                         accum_out=e2[0:116, rg:rg + 1])
                    xf = x_p.tile([116, 256], bf16, tag=f"xf_{s}_{g}")
                    nc.scalar.copy(out=xf, in_=xps)
                    xts.append(xf)
                m0 = m_p.tile([17, 256], f32, tag="mc0")
                nc.sync.dma_start(
                    out=m0,
                    in_=t0[0:1, :].rearrange("i (q c) -> i q c", q=17, c=256))
                x0ps = s2ps_p.tile([42, 256], f32, tag="x")
                nc.tensor.matmul(x0ps, a20_sb, m0, start=True, stop=True)
                ej0 = e_p.tile([42, 256], f32, tag="ejunk0")
                nc.scalar.activation(out=ej0, in_=x0ps, func=Act.Square,
                                     accum_out=e2[0:42, 32:33])
                x0f = x_p.tile([42, 256], bf16, tag=f"x0f_{s}")
                nc.scalar.copy(out=x0f, in_=x0ps)

                e2T = e_p.tile([64, 128], f32, tag="e2T")
                for a in range(4):
                    for bb in range(2):
                        nc.vector.transpose(
                            out=e2T[32 * bb:32 * bb + 32, 32 * a:32 * a + 32],
                            in_=e2[32 * a:32 * a + 32, 32 * bb:32 * bb + 32])
                ebins = e_p.tile([42, 51], f32, tag=f"eb{s}")
                nc.vector.tensor_add(ebins[0:42, 0:51], e2T[0:42, 0:51],
                                     e2T[0:42, 64:115])
                nc.vector.memset(ebins[32:33, 9:51], 1.0e30)
                nc.vector.tensor_add(ebins[32:33, 0:9], ebins[32:33, 0:9],
                                     e2T[32:33, 32:41])
                all_ebins.append(ebins)
                all_x.append((xts, x0f))

        # ================= phase 2: median =================
        with tc.tile_pool(name="mps", bufs=2, space="PSUM") as mps_p:
            P = med_p.tile([1, 4], u32)
            nc.vector.memset(P, 0)
            cnt_all = med_p.tile([42, 4], f32)
            cjunk = med_p.tile([42, 51], f32)
            cand_sb = med_p.tile([128, 4], f32)
            lsb = med_p.tile([1, 4], f32)
            stepf = med_p.tile([1, 4], f32)
            stepu = med_p.tile([1, 4], u32)
            candu = med_p.tile([1, 4], u32)
            for b in range(30, -1, -1):
                nc.vector.tensor_add(candu, P, bits[:, 4 * b:4 * b + 4])
                rps = mps_p.tile([128, 4], f32, tag="repl")
                nc.tensor.matmul(rps, o128, candu.bitcast(f32),
                                 start=True, stop=True)
                nc.vector.tensor_copy(out=cand_sb, in_=rps)
                for s in range(BL):
                    nc.vector.tensor_scalar(
                        out=cjunk, in0=all_ebins[s],
                        scalar1=cand_sb[0:42, s:s + 1], scalar2=0.0,
                        op0=Alu.is_lt, op1=Alu.add,
                        accum_out=cnt_all[0:42, s:s + 1])
                tps = mps_p.tile([2, 4], f32, tag="tot")
                nc.tensor.matmul(tps, o41, cnt_all, start=True, stop=True)
                nc.vector.tensor_scalar(out=lsb, in0=tps[0:1, :],
                                        scalar1=1024.5, scalar2=None,
                                        op0=Alu.is_lt)
                nc.vector.tensor_scalar(out=stepf, in0=lsb,
                                        scalar1=float(1 << b), scalar2=None,
                                        op0=Alu.mult)
                nc.vector.tensor_copy(out=stepu, in_=stepf)
                nc.vector.tensor_add(P, P, stepu)

            nc.sync.dma_start(out=dbgm_t.ap(), in_=P.bitcast(f32))
            nc.sync.dma_start(out=dbge_t.ap(), in_=all_ebins[0])
            den = med_p.tile([1, 4], f32)
            nc.vector.tensor_scalar(out=den, in0=P.bitcast(f32),
                                    scalar1=1.0e-6, scalar2=None, op0=Alu.add)
            dps = mps_p.tile([128, 4], f32, tag="repl")
            nc.tensor.matmul(dps, o128, den, start=True, stop=True)
            den_sb = med_p.tile([128, 4], f32)
            nc.vector.tensor_copy(out=den_sb, in_=dps)

        # ================= phase 3: mask + multiply + inverse =================
        with tc.tile_pool(name="ycls", bufs=3) as y_p, \
             tc.tile_pool(name="saps", bufs=4, space="PSUM") as saps_p, \
             tc.tile_pool(name="scls", bufs=6) as s_p, \
             tc.tile_pool(name="sturn", bufs=2) as st_p, \
             tc.tile_pool(name="sbps", bufs=2, space="PSUM") as sbps_p, \
             tc.tile_pool(name="osb", bufs=4) as o_p:
            for s in range(BL):
                ebins = all_ebins[s]
                xts, x0f = all_x[s]
                ths = e_p.tile([42, 51], f32, tag="ths")
                nc.vector.tensor_scalar(out=ths, in0=thrp,
                                        scalar1=den_sb[0:42, s:s + 1],
                                        scalar2=None, op0=Alu.mult)
                hard = e_p.tile([42, 51], f32, tag="hard")
                nc.vector.tensor_tensor(out=hard, in0=ebins, in1=ths,
                                        op=Alu.is_gt)
                md = e_p.tile([64, 64], f32, tag="madap")
                nc.vector.memset(md, 0.0)
                nc.vector.tensor_sub(md[0:42, 0:51], hard, thrp)
                nc.vector.tensor_add(md[0:42, 0:51], md[0:42, 0:51], thrp)
                mTf = e_p.tile([64, 64], f32, tag="mTf")
                for a in range(2):
                    for bb in range(2):
                        nc.vector.transpose(
                            out=mTf[32 * bb:32 * bb + 32, 32 * a:32 * a + 32],
                            in_=md[32 * a:32 * a + 32, 32 * bb:32 * bb + 32])
                mTb = e_p.tile([64, 64], bf16, tag="mTb")
                nc.vector.tensor_copy(out=mTb, in_=mTf)
                mT2 = e_p.tile([116, 42], bf16, tag="mT2")
                nc.vector.memset(mT2, 0.0)
                nc.vector.tensor_copy(out=mT2[0:51, 0:42], in_=mTb[0:51, 0:42])
                nc.vector.tensor_copy(out=mT2[64:115, 0:42],
                                      in_=mTb[0:51, 0:42])
                m0c = e_p.tile([42, 1], bf16, tag="m0c")
                nc.vector.memset(m0c, 0.0)
                nc.vector.tensor_copy(out=m0c[0:9, 0:1], in_=mTb[0:9, 32:33])
                nc.vector.tensor_copy(out=m0c[32:41, 0:1], in_=mTb[0:9, 32:33])

                st0 = st_p.tile([128, FW], bf16, tag="st0")
                st1 = st_p.tile([113, FW], bf16, tag="st1")
                for g in range(40):
                    xf = xts[g]
                    rg = g + (1 if g >= 32 else 0)
                    mre = y_p.tile([116, 256], bf16, tag="mre")
                    mim = y_p.tile([116, 256], bf16, tag="mim")
                    nc.vector.scalar_tensor_tensor(
                        out=mre, in0=whre, scalar=mT2[:, rg:rg + 1], in1=wre,
                        op0=Alu.mult, op1=Alu.add)
                    nc.vector.scalar_tensor_tensor(
                        out=mim, in0=whim, scalar=mT2[:, rg:rg + 1], in1=wim,
                        op0=Alu.mult, op1=Alu.add)
                    p1 = y_p.tile([116, 256], bf16, tag="p1")
                    p2 = y_p.tile([116, 256], bf16, tag="p2")
                    # p2 holds the sgn*X*Mim products with re/im HALVES SWAPPED
                    # so that every op below has equal input base partitions.
                    nc.vector.tensor_mul(p1, xf, mre)
                    nc.vector.scalar_tensor_tensor(
                        out=p2[0:51, :], in0=xf[64:115, :],
                        scalar=sgn2[64:115, rg:rg + 1], in1=mim[64:115, :],
                        op0=Alu.mult, op1=Alu.mult)
                    nc.vector.scalar_tensor_tensor(
                        out=p2[64:115, :], in0=xf[0:51, :],
                        scalar=sgn2[0:51, rg:rg + 1], in1=mim[0:51, :],
                        op0=Alu.mult, op1=Alu.mult)
                    yt = y_p.tile([116, 256], bf16, tag="yt")
                    nc.vector.memset(yt, 0.0)
                    nc.vector.tensor_sub(yt[0:51, :], p1[0:51, :],
                                         p2[0:51, :])
                    nc.vector.tensor_add(yt[64:115, :], p2[64:115, :],
                                         p1[64:115, :])
                    sps = saps_p.tile([116, 256], f32, tag="sa")
                    nc.tensor.matmul(sps, ainv_sb[:, 116 * g:116 * g + 116],
                                     yt, start=True, stop=True)
                    ssb = s_p.tile([116, 256], bf16, tag="s")
                    nc.scalar.copy(out=ssb, in_=sps)
                    nc.gpsimd.dma_start(
                        out=st0[3 * g + 1:3 * g + 4, :].rearrange(
                            "i (q c) -> i q c", q=17, c=256),
                        in_=ssb[0:51, :])
                    if g <= 1:
                        nc.scalar.dma_start(
                            out=st0[121 + 3 * g:124 + 3 * g, :].rearrange(
                                "i (q c) -> i q c", q=17, c=256),
                            in_=ssb[64:115, :])
                    elif g == 2:
                        nc.scalar.dma_start(
                            out=st0[127:128, :].rearrange(
                                "i (q c) -> i q c", q=17, c=256),
                            in_=ssb[64:81, :])
                        nc.scalar.dma_start(
                            out=st1[0:2, :].rearrange(
                                "i (q c) -> i q c", q=17, c=256),
                            in_=ssb[81:115, :])
                    else:
                        nc.scalar.dma_start(
                            out=st1[3 * g - 7:3 * g - 4, :].rearrange(
                                "i (q c) -> i q c", q=17, c=256),
                            in_=ssb[64:115, :])
                # class 0
                mre0 = y_p.tile([42, 256], bf16, tag="mre0")
                mim0 = y_p.tile([42, 256], bf16, tag="mim0")
                nc.vector.scalar_tensor_tensor(
                    out=mre0, in0=whre[0:42, :], scalar=m0c, in1=wre[0:42, :],
                    op0=Alu.mult, op1=Alu.add)
                nc.vector.scalar_tensor_tensor(
                    out=mim0, in0=whim[0:42, :], scalar=m0c, in1=wim[0:42, :],
                    op0=Alu.mult, op1=Alu.add)
                p10 = y_p.tile([42, 256], bf16, tag="p10")
                p20 = y_p.tile([42, 256], bf16, tag="p20")
                nc.vector.tensor_mul(p10, x0f, mre0)
                # swapped halves (see p2 above)
                nc.vector.tensor_mul(p20[0:9, :], x0f[32:41, :],
                                     mim0[32:41, :])
                nc.vector.tensor_mul(p20[32:41, :], x0f[0:9, :],
                                     mim0[0:9, :])
                y0 = y_p.tile([42, 256], bf16, tag="y0")
                nc.vector.memset(y0, 0.0)
                nc.vector.tensor_sub(y0[0:9, :], p10[0:9, :], p20[0:9, :])
                nc.vector.tensor_add(y0[32:41, :], p20[32:41, :],
                                     p10[32:41, :])
                s0ps = saps_p.tile([18, 256], f32, tag="sa")
                nc.tensor.matmul(s0ps, ainv0_sb, y0, start=True, stop=True)
                s0sb = s_p.tile([18, 256], bf16, tag="sc0")
                nc.scalar.copy(out=s0sb, in_=s0ps)
                nc.sync.dma_start(
                    out=st0[0:1, :].rearrange("i (q c) -> i q c", q=17, c=256),
                    in_=s0sb[0:17, :])

                ov = out_t.ap().rearrange("s (a b) c -> s a b c", a=241, b=17)
                for mt in range(2):
                    for fc in range(NCH):
                        lo, w = chunk(fc)
                        ps = sbps_p.tile([128, 512], f32, tag="sb")
                        nc.tensor.matmul(ps[:, 0:w],
                                         b1k0[:, 128 * mt:128 * mt + 128],
                                         st0[:, lo:lo + w],
                                         start=True, stop=False)
                        nc.tensor.matmul(ps[:, 0:w],
                                         b1k1[:, 128 * mt:128 * mt + 128],
                                         st1[:, lo:lo + w],
                                         start=False, stop=True)
                        rows = 128 if mt == 0 else 113
                        osb = o_p.tile([128, 512], f32, tag="osb")
                        nc.vector.tensor_copy(out=osb[0:rows, 0:w],
                                              in_=ps[0:rows, 0:w])
                        n2lo, n2n = lo // 256, (w + 255) // 256
                        nc.sync.dma_start(
                            out=ov[s:s + 1, 128 * mt:128 * mt + rows,
                                   n2lo:n2lo + n2n, :],
                            in_=osb[0:rows, 0:w].rearrange(
                                "p (q c) -> p q c", q=n2n, c=256))

    nc.compile()
    _NC_CACHE["nc"] = nc
    return nc


def kernel(x_in, complex_weight, complex_weight_high, threshold_param):
    from concourse.bass_utils import run_bass_kernel_spmd
    nc = _build_nc()
    bf = ml_dtypes.bfloat16

    thrp = np.asarray(threshold_param, np.float32)[_BINM.reshape(-1)]
    thrp = np.ascontiguousarray(thrp.reshape(42, 51))
    cw = np.asarray(complex_weight, np.float32)
    cwh = np.asarray(complex_weight_high, np.float32)
    wre = np.ascontiguousarray(np.broadcast_to(cw[:, 0], (116, C))).astype(bf)
    wim = np.ascontiguousarray(np.broadcast_to(cw[:, 1], (116, C))).astype(bf)
    whre = np.ascontiguousarray(np.broadcast_to(cwh[:, 0], (116, C))).astype(bf)
    whim = np.ascontiguousarray(np.broadcast_to(cwh[:, 1], (116, C))).astype(bf)

    x_in = np.ascontiguousarray(np.asarray(x_in, np.float32))
    in_maps = []
    for core in range(8):
        m = {"x": x_in[BL * core:BL * core + BL],
             "thrp": thrp, "wre": wre, "wim": wim,
             "whre": whre, "whim": whim}
        m.update(_CONSTS)
        in_maps.append(m)
    res = run_bass_kernel_spmd(nc, in_maps, core_ids=list(range(8)))
    out = np.concatenate([res.results[i]["out"] for i in range(8)], axis=0)
    return out.astype(np.float32)



# revision 3
# speedup vs baseline: 1.5330x; 1.5330x over previous
"""Adaptive Spectral Block on 8 TRN2 NeuronCores (data-parallel over batch).

N = 4097 = 241*17 Cooley-Tukey factored FFT as matmuls:
  fwd (fp16):  input cast-loaded f32->fp16, stage1 A1 [K=n1(241), M=(re|im)]
               -> t tiles fp16, corner-turn DMA -> mt [102, 256] fp16,
               stage2 A2 [102, 116] (conj-sign baked into im columns)
               -> X bins [re 0:51 | pad | im 64:115].
  spectral:    energy via ACT square-accum from PSUM (f32), 21-iter radix
               select on float bits for the median, binary mask,
               CRE/CIM coefficient tiles per group (STT), then TWO
               full-spectrum products P=X*CRE, Q=X*CIM (in big tiles).
  inv (bf16):  stageA per group: AINV2@P + AINVSWP@Q (re/im mix folded
               into the constant matrices), corner-turn2 -> [241, n2*c],
               stageB B1 -> out rows 17*n1+n2.
"""
import numpy as np
import ml_dtypes

B, N, C = 32, 4097, 256
F = N // 2 + 1
BL = B // 8
NSQ = np.sqrt(np.float64(N))
FW = 17 * C  # 4352
NCH = 9      # 8x512 + 256 free chunks
GW = 40 * C  # 10240 big-tile free width


def _build_consts():
    n1 = np.arange(241)
    k1 = np.arange(121)
    n2 = np.arange(17)
    k2 = np.arange(17)

    ang = 2 * np.pi * np.outer(n1, k1) / 241.0
    A1 = np.zeros((241, 256), np.float64)
    A1[:, 0:121] = np.cos(ang)
    A1[:, 128:248] = -np.sin(ang[:, 1:121])
    A1 /= NSQ

    def cls_mat(c):
        kk = c + 241 * k2
        th = -2 * np.pi * np.outer(n2, kk) / N
        Cm, Sm = np.cos(th), np.sin(th)
        M = np.zeros((34, 34))
        M[0:17, 0:17] = Cm
        M[17:34, 0:17] = -Sm
        M[0:17, 17:34] = Sm
        M[17:34, 17:34] = Cm
        return M

    # sign map for conjugate representative bins
    sgn = np.ones((40, 51), np.float64)
    binm = np.zeros((42, 51), np.int64)
    for g in range(40):
        rg = g + (1 if g >= 32 else 0)
        for i in range(3):
            c = 3 * g + 1 + i
            for q in range(17):
                k = c + 241 * q
                binm[rg, 17 * i + q] = k if k <= 2048 else N - k
                if k > 2048:
                    sgn[g, 17 * i + q] = -1.0
    for q in range(9):
        binm[32, q] = 241 * q

    A2_all = np.zeros((102, 40, 116), np.float64)
    for g in range(40):
        for i in range(3):
            c = 3 * g + 1 + i
            M = cls_mat(c)
            A2_all[17 * i:17 * i + 17, g, 17 * i:17 * i + 17] = M[0:17, 0:17]
            A2_all[51 + 17 * i:51 + 17 * i + 17, g, 17 * i:17 * i + 17] = M[17:34, 0:17]
            A2_all[17 * i:17 * i + 17, g, 64 + 17 * i:64 + 17 * i + 17] = M[0:17, 17:34]
            A2_all[51 + 17 * i:51 + 17 * i + 17, g, 64 + 17 * i:64 + 17 * i + 17] = M[17:34, 17:34]
    # bake conj sign into im OUTPUT columns (X'im = sgn * Xim_stored)
    for g in range(40):
        A2_all[:, g, 64:115] *= sgn[g][None, :]
    A2f = A2_all.reshape(102, 40 * 116)

    kk0 = 241 * np.arange(9)
    th0 = -2 * np.pi * np.outer(n2, kk0) / N
    A2_0 = np.zeros((17, 42), np.float64)
    A2_0[:, 0:9] = np.cos(th0)
    A2_0[:, 32:41] = np.sin(th0)

    def cls_inv(c):
        kk = c + 241 * k2
        th = +2 * np.pi * np.outer(n2, kk) / N
        Cm, Sm = np.cos(th), np.sin(th)
        M = np.zeros((34, 34))
        M[0:17, 0:17] = Cm.T
        M[17:34, 0:17] = -Sm.T
        M[0:17, 17:34] = Sm.T
        M[17:34, 17:34] = Cm.T
        return M / NSQ

    Ainv_all = np.zeros((116, 40, 116), np.float64)
    for g in range(40):
        for i in range(3):
            c = 3 * g + 1 + i
            M = cls_inv(c)
            Ainv_all[17 * i:17 * i + 17, g, 17 * i:17 * i + 17] = M[0:17, 0:17]
            Ainv_all[64 + 17 * i:64 + 17 * i + 17, g, 17 * i:17 * i + 17] = M[17:34, 0:17]
            Ainv_all[17 * i:17 * i + 17, g, 64 + 17 * i:64 + 17 * i + 17] = M[0:17, 17:34]
            Ainv_all[64 + 17 * i:64 + 17 * i + 17, g, 64 + 17 * i:64 + 17 * i + 17] = M[17:34, 17:34]

    # G3 matrices: T = AINV2 @ P + AINVSWP @ Q with
    #   P = X' * CRE, Q = X' * CIM  (X' has sgn baked into im rows)
    # yt[j]    = P[j] - Q[64+j]          (j in 0:51)
    # yt[64+j] = sgn_j * (P[64+j] + Q[j])
    AINV2 = Ainv_all.copy()
    AINVSWP = np.zeros_like(Ainv_all)
    for g in range(40):
        AINV2[64:115, g, :] = Ainv_all[64:115, g, :] * sgn[g][:, None]
        AINVSWP[0:51, g, :] = AINV2[64:115, g, :]
        AINVSWP[64:115, g, :] = -Ainv_all[0:51, g, :]
    AINV2 = AINV2.reshape(116, 40 * 116)
    AINVSWP = AINVSWP.reshape(116, 40 * 116)

    th = 2 * np.pi * np.outer(np.arange(9), n2) / 17.0
    Ainv0 = np.zeros((42, 18), np.float64)
    Ainv0[0, 0:17] = 1.0
    Ainv0[1:9, 0:17] = 2 * np.cos(th[1:9])
    Ainv0[33:41, 0:17] = -2 * np.sin(th[1:9])
    Ainv0 /= NSQ
    # class0: rows 0:9 re, 32:41 im; no conj signs
    AINV0SWP = np.zeros_like(Ainv0)
    AINV0SWP[0:9, :] = Ainv0[32:41, :]
    AINV0SWP[32:41, :] = -Ainv0[0:9, :]

    ang2 = 2 * np.pi * np.outer(k1, n1) / 241.0
    ck = np.where(k1 == 0, 1.0, 2.0)
    cosr = ck[:, None] * np.cos(ang2)
    sinr = -2.0 * np.sin(ang2[1:121])
    B1 = np.zeros((241, 256), np.float64)
    B1[0:121, 0:128] = cosr[:, 0:128]
    B1[0:121, 128:241] = cosr[:, 128:241]
    B1[121:128, 0:128] = sinr[0:7, 0:128]
    B1[121:128, 128:241] = sinr[0:7, 128:241]
    B1[128:241, 0:128] = sinr[7:120, 0:128]
    B1[128:241, 128:241] = sinr[7:120, 128:241]

    bitsr = np.zeros((1, 124), np.uint32)
    for b in range(31):
        bitsr[0, 4 * b:4 * b + 4] = np.uint32(1 << b)
    ones128 = np.ones((1, 128), np.float32)
    ones41 = np.zeros((42, 2), np.float32)
    ones41[:, 0] = 1.0

    bf = ml_dtypes.bfloat16
    f16 = np.float16
    return {
        "A1": A1.astype(f16), "A2": A2f.astype(f16),
        "A20": A2_0.astype(f16), "AINV": AINV2.astype(bf),
        "AINVS": AINVSWP.astype(bf),
        "AINV0": Ainv0.astype(bf), "AINV0S": AINV0SWP.astype(bf),
        "B1": B1.astype(bf), "BITS": bitsr,
        "ONES128": ones128, "ONES41": ones41,
    }, binm


_CONSTS, _BINM = _build_consts()
_NC_CACHE = {}


def _build_nc():
    if "nc" in _NC_CACHE:
        return _NC_CACHE["nc"]
    from contextlib import ExitStack
    from concourse import bacc, tile, mybir
    f32 = mybir.dt.float32
    f16 = mybir.dt.float16
    bf16 = mybir.dt.bfloat16
    u32 = mybir.dt.uint32
    Alu = mybir.AluOpType
    Act = mybir.ActivationFunctionType

    nc = bacc.Bacc("TRN2", target_bir_lowering=False, debug=False, num_devices=8)
    x_t = nc.dram_tensor("x", [BL, N, C], f32, kind="ExternalInput")
    thr_t = nc.dram_tensor("thrp", [42, 51], f32, kind="ExternalInput")
    wre_t = nc.dram_tensor("wre", [116, C], bf16, kind="ExternalInput")
    wim_t = nc.dram_tensor("wim", [116, C], bf16, kind="ExternalInput")
    whre_t = nc.dram_tensor("whre", [116, C], bf16, kind="ExternalInput")
    whim_t = nc.dram_tensor("whim", [116, C], bf16, kind="ExternalInput")
    a1_t = nc.dram_tensor("A1", [241, 256], f16, kind="ExternalInput")
    a2_t = nc.dram_tensor("A2", [102, 40 * 116], f16, kind="ExternalInput")
    a20_t = nc.dram_tensor("A20", [17, 42], f16, kind="ExternalInput")
    ainv_t = nc.dram_tensor("AINV", [116, 40 * 116], bf16, kind="ExternalInput")
    ainvs_t = nc.dram_tensor("AINVS", [116, 40 * 116], bf16, kind="ExternalInput")
    ainv0_t = nc.dram_tensor("AINV0", [42, 18], bf16, kind="ExternalInput")
    ainv0s_t = nc.dram_tensor("AINV0S", [42, 18], bf16, kind="ExternalInput")
    b1_t = nc.dram_tensor("B1", [241, 256], bf16, kind="ExternalInput")
    bits_t = nc.dram_tensor("BITS", [1, 124], u32, kind="ExternalInput")
    o128_t = nc.dram_tensor("ONES128", [1, 128], f32, kind="ExternalInput")
    o41_t = nc.dram_tensor("ONES41", [42, 2], f32, kind="ExternalInput")
    out_t = nc.dram_tensor("out", [BL, N, C], f32, kind="ExternalOutput")

    def chunk(fc):
        lo = fc * 512
        return lo, min(512, FW - lo)

    with tile.TileContext(nc) as tc, ExitStack() as ES:
        cpool = ES.enter_context(tc.tile_pool(name="consts", bufs=1))
        x_p = ES.enter_context(tc.tile_pool(name="xcls", bufs=1))
        e_p = ES.enter_context(tc.tile_pool(name="energy", bufs=2))
        med_p = ES.enter_context(tc.tile_pool(name="med", bufs=1))
        sp_p = ES.enter_context(tc.tile_pool(name="spect", bufs=1))

        a1k0 = cpool.tile([128, 256], f16)
        a1k1 = cpool.tile([113, 256], f16)
        nc.sync.dma_start(out=a1k0, in_=a1_t.ap()[0:128, :])
        nc.sync.dma_start(out=a1k1, in_=a1_t.ap()[128:241, :])
        a2_sb = cpool.tile([102, 40 * 116], f16)
        nc.sync.dma_start(out=a2_sb, in_=a2_t.ap())
        a20_sb = cpool.tile([17, 42], f16)
        nc.sync.dma_start(out=a20_sb, in_=a20_t.ap())
        ainv_sb = cpool.tile([116, 40 * 116], bf16)
        nc.sync.dma_start(out=ainv_sb, in_=ainv_t.ap())
        ainvs_sb = cpool.tile([116, 40 * 116], bf16)
        nc.sync.dma_start(out=ainvs_sb, in_=ainvs_t.ap())
        ainv0_sb = cpool.tile([42, 18], bf16)
        nc.sync.dma_start(out=ainv0_sb, in_=ainv0_t.ap())
        ainv0s_sb = cpool.tile([42, 18], bf16)
        nc.sync.dma_start(out=ainv0s_sb, in_=ainv0s_t.ap())
        b1k0 = cpool.tile([128, 256], bf16)
        b1k1 = cpool.tile([113, 256], bf16)
        nc.sync.dma_start(out=b1k0, in_=b1_t.ap()[0:128, :])
        nc.sync.dma_start(out=b1k1, in_=b1_t.ap()[128:241, :])
        wre = cpool.tile([116, C], bf16)
        wim = cpool.tile([116, C], bf16)
        whre = cpool.tile([116, C], bf16)
        whim = cpool.tile([116, C], bf16)
        nc.sync.dma_start(out=wre, in_=wre_t.ap())
        nc.sync.dma_start(out=wim, in_=wim_t.ap())
        nc.sync.dma_start(out=whre, in_=whre_t.ap())
        nc.sync.dma_start(out=whim, in_=whim_t.ap())
        bits = cpool.tile([1, 124], u32)
        nc.sync.dma_start(out=bits, in_=bits_t.ap())
        o128 = cpool.tile([1, 128], f32)
        nc.sync.dma_start(out=o128, in_=o128_t.ap())
        o41 = cpool.tile([42, 2], f32)
        nc.sync.dma_start(out=o41, in_=o41_t.ap())
        thrp = cpool.tile([42, 51], f32)
        nc.sync.dma_start(out=thrp, in_=thr_t.ap())

        all_ebins = []
        all_x = []

        # ================= phase 1: forward =================
        with tc.tile_pool(name="xin", bufs=1) as xin_p, \
             tc.tile_pool(name="s1ps", bufs=2, space="PSUM") as s1ps_p, \
             tc.tile_pool(name="tsb", bufs=1) as t_p, \
             tc.tile_pool(name="mt", bufs=6) as m_p, \
             tc.tile_pool(name="s2ps", bufs=4, space="PSUM") as s2ps_p:
            for s in range(BL):
                xv = x_t.ap().rearrange("s (a b) c -> s a b c", a=241, b=17)
                xin0 = xin_p.tile([128, FW], f16, tag="xin0")
                xin1 = xin_p.tile([113, FW], f16, tag="xin1")
                # cast-loads f32->fp16, spread across swdge
                nc.gpsimd.dma_start(out=xin0[0:64, :], in_=xv[s:s + 1, 0:64])
                nc.gpsimd.dma_start(out=xin0[64:128, :], in_=xv[s:s + 1, 64:128])
                nc.gpsimd.dma_start(out=xin1[0:57, :], in_=xv[s:s + 1, 128:185])
                nc.gpsimd.dma_start(out=xin1[57:113, :], in_=xv[s:s + 1, 185:241])
                t0 = t_p.tile([128, FW], f16, tag="t0")
                t1 = t_p.tile([120, FW], f16, tag="t1")
                for mt in range(2):
                    for fc in range(NCH):
                        lo, w = chunk(fc)
                        ps = s1ps_p.tile([128, 512], f32, tag="s1")
                        nc.tensor.matmul(ps[:, 0:w],
                                         a1k0[:, 128 * mt:128 * mt + 128],
                                         xin0[:, lo:lo + w],
                                         start=True, stop=False)
                        nc.tensor.matmul(ps[:, 0:w],
                                         a1k1[:, 128 * mt:128 * mt + 128],
                                         xin1[:, lo:lo + w],
                                         start=False, stop=True)
                        dst, rows = (t0, 128) if mt == 0 else (t1, 120)
                        if fc % 2 == 0:
                            nc.vector.tensor_copy(out=dst[0:rows, lo:lo + w],
                                                  in_=ps[0:rows, 0:w])
                        else:
                            nc.scalar.copy(out=dst[0:rows, lo:lo + w],
                                           in_=ps[0:rows, 0:w])

                e2 = e_p.tile([128, 64], f32, tag="e2")
                nc.vector.memset(e2, 0.0)
                nc.vector.memset(e2[0:128, 41:42], 5.0e29)
                xb = x_p.tile([116, GW], bf16, tag=f"xb_{s}")
                for g in range(40):
                    mt_g = m_p.tile([102, 256], f16, tag="m")
                    nc.scalar.dma_start(
                        out=mt_g[0:51, :],
                        in_=t0[3 * g + 1:3 * g + 4, :].rearrange(
                            "i (q c) -> i q c", q=17, c=256))
                    nc.gpsimd.dma_start(
                        out=mt_g[51:102, :],
                        in_=t1[3 * g:3 * g + 3, :].rearrange(
                            "i (q c) -> i q c", q=17, c=256))
                    xps = s2ps_p.tile([116, 256], f32, tag="x")
                    nc.tensor.matmul(xps, a2_sb[:, 116 * g:116 * g + 116],
                                     mt_g, start=True, stop=True)
                    rg = g + (1 if g >= 32 else 0)
                    ej = e_p.tile([116, 256], f32, tag="ejunk")
                    nc.scalar.activation(out=ej, in_=xps, func=Act.Square,
                                         accum_out=e2[0:116, rg:rg + 1])
                    nc.scalar.copy(out=xb[:, 256 * g:256 * g + 256], in_=xps)
                m0 = m_p.tile([17, 256], f16, tag="mc0")
                nc.sync.dma_start(
                    out=m0,
                    in_=t0[0:1, :].rearrange("i (q c) -> i q c", q=17, c=256))
                x0ps = s2ps_p.tile([42, 256], f32, tag="x")
                nc.tensor.matmul(x0ps, a20_sb, m0, start=True, stop=True)
                ej0 = e_p.tile([42, 256], f32, tag="ejunk0")
                nc.scalar.activation(out=ej0, in_=x0ps, func=Act.Square,
                                     accum_out=e2[0:42, 32:33])
                x0f = x_p.tile([42, 256], bf16, tag=f"x0f_{s}")
                nc.scalar.copy(out=x0f, in_=x0ps)

                e2T = e_p.tile([64, 128], f32, tag="e2T")
                for a in range(4):
                    for bb in range(2):
                        nc.vector.transpose(
                            out=e2T[32 * bb:32 * bb + 32, 32 * a:32 * a + 32],
                            in_=e2[32 * a:32 * a + 32, 32 * bb:32 * bb + 32])
                ebins = e_p.tile([42, 51], f32, tag=f"eb{s}")
                nc.vector.tensor_add(ebins[0:42, 0:51], e2T[0:42, 0:51],
                                     e2T[0:42, 64:115])
                nc.vector.memset(ebins[32:33, 9:51], 1.0e30)
                nc.vector.tensor_add(ebins[32:33, 0:9], ebins[32:33, 0:9],
                                     e2T[32:33, 32:41])
                all_ebins.append(ebins)
                all_x.append((xb, x0f))

        # ================= phase 2: median (bits 30..10) =================
        with tc.tile_pool(name="mps", bufs=2, space="PSUM") as mps_p:
            P = med_p.tile([1, 4], u32)
            nc.vector.memset(P, 0)
            cnt_all = med_p.tile([42, 4], f32)
            cjunk = med_p.tile([42, 51], f32)
            cand_sb = med_p.tile([128, 4], f32)
            stepf = med_p.tile([1, 4], f32)
            stepu = med_p.tile([1, 4], u32)
            candu = med_p.tile([1, 4], u32)
            for b in range(30, 9, -1):
                nc.vector.tensor_add(candu, P, bits[:, 4 * b:4 * b + 4])
                rps = mps_p.tile([128, 4], f32, tag="repl")
                nc.tensor.matmul(rps, o128, candu.bitcast(f32),
                                 start=True, stop=True)
                nc.vector.tensor_copy(out=cand_sb, in_=rps)
                for s in range(BL):
                    nc.vector.tensor_scalar(
                        out=cjunk, in0=all_ebins[s],
                        scalar1=cand_sb[0:42, s:s + 1], scalar2=0.0,
                        op0=Alu.is_lt, op1=Alu.add,
                        accum_out=cnt_all[0:42, s:s + 1])
                tps = mps_p.tile([2, 4], f32, tag="tot")
                nc.tensor.matmul(tps, o41, cnt_all, start=True, stop=True)
                nc.vector.tensor_scalar(out=stepf, in0=tps[0:1, :],
                                        scalar1=1024.5, scalar2=float(1 << b),
                                        op0=Alu.is_lt, op1=Alu.mult)
                nc.vector.tensor_copy(out=stepu, in_=stepf)
                nc.vector.tensor_add(P, P, stepu)

            den = med_p.tile([1, 4], f32)
            nc.vector.tensor_scalar(out=den, in0=P.bitcast(f32),
                                    scalar1=1.0e-6, scalar2=None, op0=Alu.add)
            dps = mps_p.tile([128, 4], f32, tag="repl")
            nc.tensor.matmul(dps, o128, den, start=True, stop=True)
            den_sb = med_p.tile([128, 4], f32)
            nc.vector.tensor_copy(out=den_sb, in_=dps)

        # ================= phase 3: mask + products + inverse =================
        with tc.tile_pool(name="y0cls", bufs=2) as y_p, \
             tc.tile_pool(name="saps", bufs=4, space="PSUM") as saps_p, \
             tc.tile_pool(name="scls", bufs=4) as s_p, \
             tc.tile_pool(name="sturn", bufs=1) as st_p, \
             tc.tile_pool(name="sbps", bufs=2, space="PSUM") as sbps_p, \
             tc.tile_pool(name="osb", bufs=4) as o_p:
            for s in range(BL):
                ebins = all_ebins[s]
                xb, x0f = all_x[s]
                ths = e_p.tile([42, 51], f32, tag="ths")
                nc.vector.tensor_scalar(out=ths, in0=thrp,
                                        scalar1=den_sb[0:42, s:s + 1],
                                        scalar2=None, op0=Alu.mult)
                hard = e_p.tile([42, 51], f32, tag="hard")
                nc.vector.tensor_tensor(out=hard, in0=ebins, in1=ths,
                                        op=Alu.is_gt)
                md = e_p.tile([64, 64], f32, tag="madap")
                nc.vector.memset(md, 0.0)
                nc.vector.tensor_sub(md[0:42, 0:51], hard, thrp)
                nc.vector.tensor_add(md[0:42, 0:51], md[0:42, 0:51], thrp)
                mTf = e_p.tile([64, 64], f32, tag="mTf")
                for a in range(2):
                    for bb in range(2):
                        nc.vector.transpose(
                            out=mTf[32 * bb:32 * bb + 32, 32 * a:32 * a + 32],
                            in_=md[32 * a:32 * a + 32, 32 * bb:32 * bb + 32])
                mTb = e_p.tile([64, 64], bf16, tag="mTb")
                nc.vector.tensor_copy(out=mTb, in_=mTf)
                mT2 = e_p.tile([116, 42], bf16, tag="mT2")
                nc.vector.memset(mT2, 0.0)
                nc.vector.tensor_copy(out=mT2[0:51, 0:42], in_=mTb[0:51, 0:42])
                nc.vector.tensor_copy(out=mT2[64:115, 0:42],
                                      in_=mTb[0:51, 0:42])
                m0c = e_p.tile([42, 1], bf16, tag="m0c")
                nc.vector.memset(m0c, 0.0)
                nc.vector.tensor_copy(out=m0c[0:9, 0:1], in_=mTb[0:9, 32:33])
                nc.vector.tensor_copy(out=m0c[32:41, 0:1], in_=mTb[0:9, 32:33])

                # CRE/CIM coefficient big tiles
                cre = sp_p.tile([116, GW], bf16, tag="cre")
                cim = sp_p.tile([116, GW], bf16, tag="cim")
                for g in range(40):
                    rg = g + (1 if g >= 32 else 0)
                    nc.vector.scalar_tensor_tensor(
                        out=cre[:, 256 * g:256 * g + 256], in0=whre,
                        scalar=mT2[:, rg:rg + 1], in1=wre,
                        op0=Alu.mult, op1=Alu.add)
                    nc.vector.scalar_tensor_tensor(
                        out=cim[:, 256 * g:256 * g + 256], in0=whim,
                        scalar=mT2[:, rg:rg + 1], in1=wim,
                        op0=Alu.mult, op1=Alu.add)
                # in-place products: P = X*CRE, Q = X*CIM
                nc.vector.tensor_tensor(out=cre, in0=xb, in1=cre, op=Alu.mult)
                nc.vector.tensor_tensor(out=cim, in0=xb, in1=cim, op=Alu.mult)

                st0 = st_p.tile([128, FW], bf16, tag="st0")
                st1 = st_p.tile([113, FW], bf16, tag="st1")
                for g in range(40):
                    sps = saps_p.tile([116, 256], f32, tag="sa")
                    nc.tensor.matmul(sps, ainv_sb[:, 116 * g:116 * g + 116],
                                     cre[:, 256 * g:256 * g + 256],
                                     start=True, stop=False)
                    nc.tensor.matmul(sps, ainvs_sb[:, 116 * g:116 * g + 116],
                                     cim[:, 256 * g:256 * g + 256],
                                     start=False, stop=True)
                    ssb = s_p.tile([116, 256], bf16, tag="s")
                    nc.scalar.copy(out=ssb, in_=sps)
                    nc.gpsimd.dma_start(
                        out=st0[3 * g + 1:3 * g + 4, :].rearrange(
                            "i (q c) -> i q c", q=17, c=256),
                        in_=ssb[0:51, :])
                    if g <= 1:
                        nc.scalar.dma_start(
                            out=st0[121 + 3 * g:124 + 3 * g, :].rearrange(
                                "i (q c) -> i q c", q=17, c=256),
                            in_=ssb[64:115, :])
                    elif g == 2:
                        nc.scalar.dma_start(
                            out=st0[127:128, :].rearrange(
                                "i (q c) -> i q c", q=17, c=256),
                            in_=ssb[64:81, :])
                        nc.scalar.dma_start(
                            out=st1[0:2, :].rearrange(
                                "i (q c) -> i q c", q=17, c=256),
                            in_=ssb[81:115, :])
                    else:
                        nc.scalar.dma_start(
                            out=st1[3 * g - 7:3 * g - 4, :].rearrange(
                                "i (q c) -> i q c", q=17, c=256),
                            in_=ssb[64:115, :])
                # class 0
                cre0 = y_p.tile([42, 256], bf16, tag="cre0")
                cim0 = y_p.tile([42, 256], bf16, tag="cim0")
                nc.vector.scalar_tensor_tensor(
                    out=cre0, in0=whre[0:42, :], scalar=m0c, in1=wre[0:42, :],
                    op0=Alu.mult, op1=Alu.add)
                nc.vector.scalar_tensor_tensor(
                    out=cim0, in0=whim[0:42, :], scalar=m0c, in1=wim[0:42, :],
                    op0=Alu.mult, op1=Alu.add)
                nc.vector.tensor_tensor(out=cre0, in0=x0f, in1=cre0, op=Alu.mult)
                nc.vector.tensor_tensor(out=cim0, in0=x0f, in1=cim0, op=Alu.mult)
                s0ps = saps_p.tile([18, 256], f32, tag="sa")
                nc.tensor.matmul(s0ps, ainv0_sb, cre0, start=True, stop=False)
                nc.tensor.matmul(s0ps, ainv0s_sb, cim0, start=False, stop=True)
                s0sb = s_p.tile([18, 256], bf16, tag="sc0")
                nc.scalar.copy(out=s0sb, in_=s0ps)
                nc.sync.dma_start(
                    out=st0[0:1, :].rearrange("i (q c) -> i q c", q=17, c=256),
                    in_=s0sb[0:17, :])

                ov = out_t.ap().rearrange("s (a b) c -> s a b c", a=241, b=17)
                for mt in range(2):
                    for fc in range(NCH):
                        lo, w = chunk(fc)
                        ps = sbps_p.tile([128, 512], f32, tag="sb")
                        nc.tensor.matmul(ps[:, 0:w],
                                         b1k0[:, 128 * mt:128 * mt + 128],
                                         st0[:, lo:lo + w],
                                         start=True, stop=False)
                        nc.tensor.matmul(ps[:, 0:w],
                                         b1k1[:, 128 * mt:128 * mt + 128],
                                         st1[:, lo:lo + w],
                                         start=False, stop=True)
                        rows = 128 if mt == 0 else 113
                        osb = o_p.tile([128, 512], f32, tag="osb")
                        if fc % 2 == 0:
                            nc.vector.tensor_copy(out=osb[0:rows, 0:w],
                                                  in_=ps[0:rows, 0:w])
                        else:
                            nc.scalar.copy(out=osb[0:rows, 0:w],
                                           in_=ps[0:rows, 0:w])
                        n2lo, n2n = lo // 256, (w + 255) // 256
                        nc.sync.dma_start(
                            out=ov[s:s + 1, 128 * mt:128 * mt + rows,
                                   n2lo:n2lo + n2n, :],
                            in_=osb[0:rows, 0:w].rearrange(
                                "p (q c) -> p q c", q=n2n, c=256))

    nc.compile()
    _NC_CACHE["nc"] = nc
    return nc


def kernel(x_in, complex_weight, complex_weight_high, threshold_param):
    from concourse.bass_utils import run_bass_kernel_spmd
    nc = _build_nc()
    bf = ml_dtypes.bfloat16

    thrp = np.asarray(threshold_param, np.float32)[_BINM.reshape(-1)]
    thrp = np.ascontiguousarray(thrp.reshape(42, 51))
    cw = np.asarray(complex_weight, np.float32)
    cwh = np.asarray(complex_weight_high, np.float32)
    wre = np.ascontiguousarray(np.broadcast_to(cw[:, 0], (116, C))).astype(bf)
    wim = np.ascontiguousarray(np.broadcast_to(cw[:, 1], (116, C))).astype(bf)
    whre = np.ascontiguousarray(np.broadcast_to(cwh[:, 0], (116, C))).astype(bf)
    whim = np.ascontiguousarray(np.broadcast_to(cwh[:, 1], (116, C))).astype(bf)

    x_in = np.ascontiguousarray(np.asarray(x_in, np.float32))
    in_maps = []
    for core in range(8):
        m = {"x": x_in[BL * core:BL * core + BL],
             "thrp": thrp, "wre": wre, "wim": wim,
             "whre": whre, "whim": whim}
        m.update(_CONSTS)
        in_maps.append(m)
    res = run_bass_kernel_spmd(nc, in_maps, core_ids=list(range(8)))
    out = np.concatenate([res.results[i]["out"] for i in range(8)], axis=0)
    return out.astype(np.float32)


# revision 6
# speedup vs baseline: 1.7255x; 1.1256x over previous
"""Adaptive Spectral Block on 8 TRN2 NeuronCores (data-parallel over batch).

N = 4097 = 241*17 Cooley-Tukey factored FFT as matmuls:
  fwd (fp16):  input cast-loaded f32->fp16, stage1 A1 [K=n1(241), M=(re|im)]
               -> t tiles fp16, corner-turn DMA -> mt [102, 256] fp16,
               stage2 A2 [102, 116] (conj-sign baked into im columns)
               -> X bins [re 0:51 | pad | im 64:115].
  spectral:    energy via ACT square-accum from PSUM (f32), 21-iter radix
               select on float bits for the median, binary mask,
               CRE/CIM coefficient tiles per group (STT), then TWO
               full-spectrum products P=X*CRE, Q=X*CIM (in big tiles).
  inv (bf16):  stageA per group: AINV2@P + AINVSWP@Q (re/im mix folded
               into the constant matrices), corner-turn2 -> [241, n2*c],
               stageB B1 -> out rows 17*n1+n2.
"""
import numpy as np
import ml_dtypes

B, N, C = 32, 4097, 256
F = N // 2 + 1
BL = B // 8
NSQ = np.sqrt(np.float64(N))
FW = 17 * C  # 4352
NCH = 9      # 8x512 + 256 free chunks
GW = 40 * C  # 10240 big-tile free width


def _build_consts():
    n1 = np.arange(241)
    k1 = np.arange(121)
    n2 = np.arange(17)
    k2 = np.arange(17)

    ang = 2 * np.pi * np.outer(n1, k1) / 241.0
    A1 = np.zeros((241, 256), np.float64)
    A1[:, 0:121] = np.cos(ang)
    A1[:, 128:248] = -np.sin(ang[:, 1:121])
    A1 /= NSQ

    def cls_mat(c):
        kk = c + 241 * k2
        th = -2 * np.pi * np.outer(n2, kk) / N
        Cm, Sm = np.cos(th), np.sin(th)
        M = np.zeros((34, 34))
        M[0:17, 0:17] = Cm
        M[17:34, 0:17] = -Sm
        M[0:17, 17:34] = Sm
        M[17:34, 17:34] = Cm
        return M

    # sign map for conjugate representative bins
    sgn = np.ones((40, 51), np.float64)
    binm = np.zeros((42, 51), np.int64)
    for g in range(40):
        rg = g + (1 if g >= 32 else 0)
        for i in range(3):
            c = 3 * g + 1 + i
            for q in range(17):
                k = c + 241 * q
                binm[rg, 17 * i + q] = k if k <= 2048 else N - k
                if k > 2048:
                    sgn[g, 17 * i + q] = -1.0
    for q in range(9):
        binm[32, q] = 241 * q

    A2_all = np.zeros((102, 40, 116), np.float64)
    for g in range(40):
        for i in range(3):
            c = 3 * g + 1 + i
            M = cls_mat(c)
            A2_all[17 * i:17 * i + 17, g, 17 * i:17 * i + 17] = M[0:17, 0:17]
            A2_all[51 + 17 * i:51 + 17 * i + 17, g, 17 * i:17 * i + 17] = M[17:34, 0:17]
            A2_all[17 * i:17 * i + 17, g, 64 + 17 * i:64 + 17 * i + 17] = M[0:17, 17:34]
            A2_all[51 + 17 * i:51 + 17 * i + 17, g, 64 + 17 * i:64 + 17 * i + 17] = M[17:34, 17:34]
    # bake conj sign into im OUTPUT columns (X'im = sgn * Xim_stored)
    for g in range(40):
        A2_all[:, g, 64:115] *= sgn[g][None, :]
    A2f = A2_all.reshape(102, 40 * 116)

    kk0 = 241 * np.arange(9)
    th0 = -2 * np.pi * np.outer(n2, kk0) / N
    A2_0 = np.zeros((17, 42), np.float64)
    A2_0[:, 0:9] = np.cos(th0)
    A2_0[:, 32:41] = np.sin(th0)

    def cls_inv(c):
        kk = c + 241 * k2
        th = +2 * np.pi * np.outer(n2, kk) / N
        Cm, Sm = np.cos(th), np.sin(th)
        M = np.zeros((34, 34))
        M[0:17, 0:17] = Cm.T
        M[17:34, 0:17] = -Sm.T
        M[0:17, 17:34] = Sm.T
        M[17:34, 17:34] = Cm.T
        return M / NSQ

    Ainv_all = np.zeros((116, 40, 116), np.float64)
    for g in range(40):
        for i in range(3):
            c = 3 * g + 1 + i
            M = cls_inv(c)
            Ainv_all[17 * i:17 * i + 17, g, 17 * i:17 * i + 17] = M[0:17, 0:17]
            Ainv_all[64 + 17 * i:64 + 17 * i + 17, g, 17 * i:17 * i + 17] = M[17:34, 0:17]
            Ainv_all[17 * i:17 * i + 17, g, 64 + 17 * i:64 + 17 * i + 17] = M[0:17, 17:34]
            Ainv_all[64 + 17 * i:64 + 17 * i + 17, g, 64 + 17 * i:64 + 17 * i + 17] = M[17:34, 17:34]

    # G3 matrices: T = AINV2 @ P + AINVSWP @ Q with
    #   P = X' * CRE, Q = X' * CIM  (X' has sgn baked into im rows)
    # yt[j]    = P[j] - Q[64+j]          (j in 0:51)
    # yt[64+j] = sgn_j * (P[64+j] + Q[j])
    AINV2 = Ainv_all.copy()
    AINVSWP = np.zeros_like(Ainv_all)
    for g in range(40):
        AINV2[64:115, g, :] = Ainv_all[64:115, g, :] * sgn[g][:, None]
        AINVSWP[0:51, g, :] = AINV2[64:115, g, :]
        AINVSWP[64:115, g, :] = -Ainv_all[0:51, g, :]
    AINV2 = AINV2.reshape(116, 40 * 116)
    AINVSWP = AINVSWP.reshape(116, 40 * 116)

    th = 2 * np.pi * np.outer(np.arange(9), n2) / 17.0
    Ainv0 = np.zeros((42, 18), np.float64)
    Ainv0[0, 0:17] = 1.0
    Ainv0[1:9, 0:17] = 2 * np.cos(th[1:9])
    Ainv0[33:41, 0:17] = -2 * np.sin(th[1:9])
    Ainv0 /= NSQ
    # class0: rows 0:9 re, 32:41 im; no conj signs
    AINV0SWP = np.zeros_like(Ainv0)
    AINV0SWP[0:9, :] = Ainv0[32:41, :]
    AINV0SWP[32:41, :] = -Ainv0[0:9, :]

    ang2 = 2 * np.pi * np.outer(k1, n1) / 241.0
    ck = np.where(k1 == 0, 1.0, 2.0)
    cosr = ck[:, None] * np.cos(ang2)
    sinr = -2.0 * np.sin(ang2[1:121])
    B1 = np.zeros((241, 256), np.float64)
    B1[0:121, 0:128] = cosr[:, 0:128]
    B1[0:121, 128:241] = cosr[:, 128:241]
    B1[121:128, 0:128] = sinr[0:7, 0:128]
    B1[121:128, 128:241] = sinr[0:7, 128:241]
    B1[128:241, 0:128] = sinr[7:120, 0:128]
    B1[128:241, 128:241] = sinr[7:120, 128:241]

    bitsr = np.zeros((1, 124), np.uint32)
    for b in range(31):
        bitsr[0, 4 * b:4 * b + 4] = np.uint32(1 << b)
    ones128 = np.ones((1, 128), np.float32)
    ones41 = np.zeros((42, 2), np.float32)
    ones41[:, 0] = 1.0

    # ---- class-interleaved permutations: 1 corner-turn DMA per group ----
    # t0 rows [127]: 0 = class0-cos; class c in 1..63: 2c-1 = cos/re, 2c = im
    # t1 rows [114]: class c in 64..120: 2(c-64) = re, +1 = im
    A1P = np.zeros((241, 256), np.float64)
    A1P[:, 0] = A1[:, 0]
    for c in range(1, 64):
        A1P[:, 2 * c - 1] = A1[:, c]
        A1P[:, 2 * c] = A1[:, 127 + c]
    for c in range(64, 121):
        A1P[:, 128 + 2 * (c - 64)] = A1[:, c]
        A1P[:, 128 + 2 * (c - 64) + 1] = A1[:, 127 + c]

    # mt rows [102]: 34i + 17ri + q  (was re 17i+q / im 51+17i+q)
    oldidx = np.zeros(102, np.int64)
    for i in range(3):
        for q in range(17):
            oldidx[34 * i + q] = 17 * i + q
            oldidx[34 * i + 17 + q] = 51 + 17 * i + q
    A2P = A2f[oldidx]

    # Ainv M-columns -> interleaved compact [102]: 34i+17cs+q
    # (was cos 17i+q, sin 64+17i+q); also st rows match t rows pattern
    mold = np.zeros(102, np.int64)
    for i in range(3):
        for q in range(17):
            mold[34 * i + q] = 17 * i + q
            mold[34 * i + 17 + q] = 64 + 17 * i + q
    AINVP = AINV2.reshape(116, 40, 116)[:, :, mold].reshape(116, 40 * 102)
    AINVSP = AINVSWP.reshape(116, 40, 116)[:, :, mold].reshape(116, 40 * 102)

    # B1 rows permuted to st layout: [0:127] = st0 rows, [127:241] = st1 rows
    B1P = np.zeros((241, 256), np.float64)
    B1P[0] = B1[0]
    for c in range(1, 64):
        B1P[2 * c - 1] = B1[c]
        B1P[2 * c] = B1[120 + c]
    for c in range(64, 121):
        B1P[127 + 2 * (c - 64)] = B1[c]
        B1P[127 + 2 * (c - 64) + 1] = B1[120 + c]

    bf = ml_dtypes.bfloat16
    f16 = np.float16
    return {
        "A1": A1P.astype(f16), "A2": A2P.astype(f16),
        "A20": A2_0.astype(f16), "AINV": AINVP.astype(bf),
        "AINVS": AINVSP.astype(bf),
        "AINV0": Ainv0.astype(bf), "AINV0S": AINV0SWP.astype(bf),
        "B1": B1P.astype(bf), "BITS": bitsr,
        "ONES128": ones128, "ONES41": ones41,
    }, binm


_CONSTS, _BINM = _build_consts()
_NC_CACHE = {}


def _build_nc():
    if "nc" in _NC_CACHE:
        return _NC_CACHE["nc"]
    from contextlib import ExitStack
    from concourse import bacc, tile, mybir
    f32 = mybir.dt.float32
    f16 = mybir.dt.float16
    bf16 = mybir.dt.bfloat16
    u32 = mybir.dt.uint32
    Alu = mybir.AluOpType
    Act = mybir.ActivationFunctionType

    nc = bacc.Bacc("TRN2", target_bir_lowering=False, debug=False, num_devices=8)
    x_t = nc.dram_tensor("x", [BL, N, C], f32, kind="ExternalInput")
    thr_t = nc.dram_tensor("thrp", [42, 51], f32, kind="ExternalInput")
    wre_t = nc.dram_tensor("wre", [116, C], bf16, kind="ExternalInput")
    wim_t = nc.dram_tensor("wim", [116, C], bf16, kind="ExternalInput")
    whre_t = nc.dram_tensor("whre", [116, C], bf16, kind="ExternalInput")
    whim_t = nc.dram_tensor("whim", [116, C], bf16, kind="ExternalInput")
    a1_t = nc.dram_tensor("A1", [241, 256], f16, kind="ExternalInput")
    a2_t = nc.dram_tensor("A2", [102, 40 * 116], f16, kind="ExternalInput")
    a20_t = nc.dram_tensor("A20", [17, 42], f16, kind="ExternalInput")
    ainv_t = nc.dram_tensor("AINV", [116, 40 * 102], bf16, kind="ExternalInput")
    ainvs_t = nc.dram_tensor("AINVS", [116, 40 * 102], bf16, kind="ExternalInput")
    ainv0_t = nc.dram_tensor("AINV0", [42, 18], bf16, kind="ExternalInput")
    ainv0s_t = nc.dram_tensor("AINV0S", [42, 18], bf16, kind="ExternalInput")
    b1_t = nc.dram_tensor("B1", [241, 256], bf16, kind="ExternalInput")
    bits_t = nc.dram_tensor("BITS", [1, 124], u32, kind="ExternalInput")
    o128_t = nc.dram_tensor("ONES128", [1, 128], f32, kind="ExternalInput")
    o41_t = nc.dram_tensor("ONES41", [42, 2], f32, kind="ExternalInput")
    out_t = nc.dram_tensor("out", [BL, N, C], f32, kind="ExternalOutput")

    def chunk(fc):
        lo = fc * 512
        return lo, min(512, FW - lo)

    with tile.TileContext(nc) as tc, ExitStack() as ES:
        cpool = ES.enter_context(tc.tile_pool(name="consts", bufs=1))
        x_p = ES.enter_context(tc.tile_pool(name="xcls", bufs=1))
        e_p = ES.enter_context(tc.tile_pool(name="energy", bufs=2))
        med_p = ES.enter_context(tc.tile_pool(name="med", bufs=1))
        sp_p = ES.enter_context(tc.tile_pool(name="spect", bufs=1))

        a1k0 = cpool.tile([128, 256], f16)
        a1k1 = cpool.tile([113, 256], f16)
        nc.sync.dma_start(out=a1k0, in_=a1_t.ap()[0:128, :])
        nc.sync.dma_start(out=a1k1, in_=a1_t.ap()[128:241, :])
        a2_sb = cpool.tile([102, 40 * 116], f16)
        nc.sync.dma_start(out=a2_sb, in_=a2_t.ap())
        a20_sb = cpool.tile([17, 42], f16)
        nc.sync.dma_start(out=a20_sb, in_=a20_t.ap())
        ainv_sb = cpool.tile([116, 40 * 102], bf16)
        nc.sync.dma_start(out=ainv_sb, in_=ainv_t.ap())
        ainvs_sb = cpool.tile([116, 40 * 102], bf16)
        nc.sync.dma_start(out=ainvs_sb, in_=ainvs_t.ap())
        ainv0_sb = cpool.tile([42, 18], bf16)
        nc.sync.dma_start(out=ainv0_sb, in_=ainv0_t.ap())
        ainv0s_sb = cpool.tile([42, 18], bf16)
        nc.sync.dma_start(out=ainv0s_sb, in_=ainv0s_t.ap())
        b1k0 = cpool.tile([127, 256], bf16)
        b1k1 = cpool.tile([114, 256], bf16)
        nc.sync.dma_start(out=b1k0, in_=b1_t.ap()[0:127, :])
        nc.sync.dma_start(out=b1k1, in_=b1_t.ap()[127:241, :])
        wre = cpool.tile([116, C], bf16)
        wim = cpool.tile([116, C], bf16)
        whre = cpool.tile([116, C], bf16)
        whim = cpool.tile([116, C], bf16)
        nc.sync.dma_start(out=wre, in_=wre_t.ap())
        nc.sync.dma_start(out=wim, in_=wim_t.ap())
        nc.sync.dma_start(out=whre, in_=whre_t.ap())
        nc.sync.dma_start(out=whim, in_=whim_t.ap())
        bits = cpool.tile([1, 124], u32)
        nc.sync.dma_start(out=bits, in_=bits_t.ap())
        o128 = cpool.tile([1, 128], f32)
        nc.sync.dma_start(out=o128, in_=o128_t.ap())
        o41 = cpool.tile([42, 2], f32)
        nc.sync.dma_start(out=o41, in_=o41_t.ap())
        thrp = cpool.tile([42, 51], f32)
        nc.sync.dma_start(out=thrp, in_=thr_t.ap())

        all_ebins = []
        all_x = []

        # ================= phase 1: forward =================
        with tc.tile_pool(name="xin", bufs=1) as xin_p, \
             tc.tile_pool(name="s1ps", bufs=2, space="PSUM") as s1ps_p, \
             tc.tile_pool(name="tsb", bufs=1) as t_p, \
             tc.tile_pool(name="mt", bufs=6) as m_p, \
             tc.tile_pool(name="s2ps", bufs=4, space="PSUM") as s2ps_p:
            for s in range(BL):
                xv = x_t.ap().rearrange("s (a b) c -> s a b c", a=241, b=17)
                xin0 = xin_p.tile([128, FW], f16, tag="xin0")
                xin1 = xin_p.tile([113, FW], f16, tag="xin1")
                # cast-loads f32->fp16, spread across swdge
                nc.gpsimd.dma_start(out=xin0[0:64, :], in_=xv[s:s + 1, 0:64])
                nc.gpsimd.dma_start(out=xin0[64:128, :], in_=xv[s:s + 1, 64:128])
                nc.gpsimd.dma_start(out=xin1[0:57, :], in_=xv[s:s + 1, 128:185])
                nc.gpsimd.dma_start(out=xin1[57:113, :], in_=xv[s:s + 1, 185:241])
                t0 = t_p.tile([128, FW], f16, tag="t0")
                t1 = t_p.tile([114, FW], f16, tag="t1")
                for mt in range(2):
                    for fc in range(NCH):
                        lo, w = chunk(fc)
                        ps = s1ps_p.tile([128, 512], f32, tag="s1")
                        nc.tensor.matmul(ps[:, 0:w],
                                         a1k0[:, 128 * mt:128 * mt + 128],
                                         xin0[:, lo:lo + w],
                                         start=True, stop=False)
                        nc.tensor.matmul(ps[:, 0:w],
                                         a1k1[:, 128 * mt:128 * mt + 128],
                                         xin1[:, lo:lo + w],
                                         start=False, stop=True)
                        dst, rows = (t0, 128) if mt == 0 else (t1, 114)
                        if fc % 2 == 0:
                            nc.vector.tensor_copy(out=dst[0:rows, lo:lo + w],
                                                  in_=ps[0:rows, 0:w])
                        else:
                            nc.scalar.copy(out=dst[0:rows, lo:lo + w],
                                           in_=ps[0:rows, 0:w])

                e2 = e_p.tile([128, 64], f32, tag="e2")
                nc.vector.memset(e2, 0.0)
                nc.vector.memset(e2[0:128, 41:42], 5.0e29)
                xb = x_p.tile([116, GW], bf16, tag=f"xb_{s}")
                for g in range(40):
                    mt_g = m_p.tile([102, 256], f16, tag="m")
                    ctsrc = (t0[6 * g + 1:6 * g + 7, :] if g <= 20 else
                             t1[6 * (g - 21):6 * (g - 21) + 6, :]).rearrange(
                                 "i (q c) -> i q c", q=17, c=256)
                    cteng = (nc.sync, nc.gpsimd, nc.scalar)[g % 3]
                    cteng.dma_start(out=mt_g, in_=ctsrc)
                    xps = s2ps_p.tile([116, 256], f32, tag="x")
                    nc.tensor.matmul(xps, a2_sb[:, 116 * g:116 * g + 116],
                                     mt_g, start=True, stop=True)
                    rg = g + (1 if g >= 32 else 0)
                    ej = e_p.tile([116, 256], f32, tag="ejunk")
                    nc.scalar.activation(out=ej, in_=xps, func=Act.Square,
                                         accum_out=e2[0:116, rg:rg + 1])
                    nc.vector.tensor_copy(out=xb[:, 256 * g:256 * g + 256],
                                          in_=xps)
                m0 = m_p.tile([17, 256], f16, tag="mc0")
                nc.sync.dma_start(
                    out=m0,
                    in_=t0[0:1, :].rearrange("i (q c) -> i q c", q=17, c=256))
                x0ps = s2ps_p.tile([42, 256], f32, tag="x")
                nc.tensor.matmul(x0ps, a20_sb, m0, start=True, stop=True)
                ej0 = e_p.tile([42, 256], f32, tag="ejunk0")
                nc.scalar.activation(out=ej0, in_=x0ps, func=Act.Square,
                                     accum_out=e2[0:42, 32:33])
                x0f = x_p.tile([42, 256], bf16, tag=f"x0f_{s}")
                nc.scalar.copy(out=x0f, in_=x0ps)

                e2T = e_p.tile([64, 128], f32, tag="e2T")
                for a in range(4):
                    for bb in range(2):
                        nc.vector.transpose(
                            out=e2T[32 * bb:32 * bb + 32, 32 * a:32 * a + 32],
                            in_=e2[32 * a:32 * a + 32, 32 * bb:32 * bb + 32])
                ebins = e_p.tile([42, 51], f32, tag=f"eb{s}")
                nc.vector.tensor_add(ebins[0:42, 0:51], e2T[0:42, 0:51],
                                     e2T[0:42, 64:115])
                nc.vector.memset(ebins[32:33, 9:51], 1.0e30)
                nc.vector.tensor_add(ebins[32:33, 0:9], ebins[32:33, 0:9],
                                     e2T[32:33, 32:41])
                all_ebins.append(ebins)
                all_x.append((xb, x0f))

        # ================= phase 2: median (bits 30..10) =================
        with tc.tile_pool(name="mps", bufs=2, space="PSUM") as mps_p:
            P = med_p.tile([1, 4], u32)
            nc.vector.memset(P, 0)
            cnt_all = med_p.tile([42, 4], f32)
            cjunk = med_p.tile([42, 51], f32)
            cand_sb = med_p.tile([128, 4], f32)
            stepf = med_p.tile([1, 4], f32)
            stepu = med_p.tile([1, 4], u32)
            candu = med_p.tile([1, 4], u32)
            for b in range(30, 9, -1):
                nc.vector.tensor_add(candu, P, bits[:, 4 * b:4 * b + 4])
                rps = mps_p.tile([128, 4], f32, tag="repl")
                nc.tensor.matmul(rps, o128, candu.bitcast(f32),
                                 start=True, stop=True)
                nc.vector.tensor_copy(out=cand_sb, in_=rps)
                for s in range(BL):
                    nc.vector.tensor_scalar(
                        out=cjunk, in0=all_ebins[s],
                        scalar1=cand_sb[0:42, s:s + 1], scalar2=0.0,
                        op0=Alu.is_lt, op1=Alu.add,
                        accum_out=cnt_all[0:42, s:s + 1])
                tps = mps_p.tile([2, 4], f32, tag="tot")
                nc.tensor.matmul(tps, o41, cnt_all, start=True, stop=True)
                nc.vector.tensor_scalar(out=stepf, in0=tps[0:1, :],
                                        scalar1=1024.5, scalar2=float(1 << b),
                                        op0=Alu.is_lt, op1=Alu.mult)
                nc.vector.tensor_copy(out=stepu, in_=stepf)
                nc.vector.tensor_add(P, P, stepu)

            den = med_p.tile([1, 4], f32)
            nc.vector.tensor_scalar(out=den, in0=P.bitcast(f32),
                                    scalar1=1.0e-6, scalar2=None, op0=Alu.add)
            dps = mps_p.tile([128, 4], f32, tag="repl")
            nc.tensor.matmul(dps, o128, den, start=True, stop=True)
            den_sb = med_p.tile([128, 4], f32)
            nc.vector.tensor_copy(out=den_sb, in_=dps)

        # ================= phase 3: mask + products + inverse =================
        with tc.tile_pool(name="y0cls", bufs=2) as y_p, \
             tc.tile_pool(name="saps", bufs=4, space="PSUM") as saps_p, \
             tc.tile_pool(name="scls", bufs=4) as s_p, \
             tc.tile_pool(name="sturn", bufs=1) as st_p, \
             tc.tile_pool(name="sbps", bufs=2, space="PSUM") as sbps_p, \
             tc.tile_pool(name="osb", bufs=4) as o_p:
            for s in range(BL):
                ebins = all_ebins[s]
                xb, x0f = all_x[s]
                ths = e_p.tile([42, 51], f32, tag="ths")
                nc.vector.tensor_scalar(out=ths, in0=thrp,
                                        scalar1=den_sb[0:42, s:s + 1],
                                        scalar2=None, op0=Alu.mult)
                hard = e_p.tile([42, 51], f32, tag="hard")
                nc.vector.tensor_tensor(out=hard, in0=ebins, in1=ths,
                                        op=Alu.is_gt)
                md = e_p.tile([64, 64], f32, tag="madap")
                nc.vector.memset(md, 0.0)
                nc.vector.tensor_sub(md[0:42, 0:51], hard, thrp)
                nc.vector.tensor_add(md[0:42, 0:51], md[0:42, 0:51], thrp)
                mTf = e_p.tile([64, 64], f32, tag="mTf")
                for a in range(2):
                    for bb in range(2):
                        nc.vector.transpose(
                            out=mTf[32 * bb:32 * bb + 32, 32 * a:32 * a + 32],
                            in_=md[32 * a:32 * a + 32, 32 * bb:32 * bb + 32])
                mTb = e_p.tile([64, 64], bf16, tag="mTb")
                nc.vector.tensor_copy(out=mTb, in_=mTf)
                mT2 = e_p.tile([116, 42], bf16, tag="mT2")
                nc.vector.memset(mT2, 0.0)
                nc.vector.tensor_copy(out=mT2[0:51, 0:42], in_=mTb[0:51, 0:42])
                nc.vector.tensor_copy(out=mT2[64:115, 0:42],
                                      in_=mTb[0:51, 0:42])
                m0c = e_p.tile([42, 1], bf16, tag="m0c")
                nc.vector.memset(m0c, 0.0)
                nc.vector.tensor_copy(out=m0c[0:9, 0:1], in_=mTb[0:9, 32:33])
                nc.vector.tensor_copy(out=m0c[32:41, 0:1], in_=mTb[0:9, 32:33])

                # CRE/CIM coefficient big tiles
                cre = sp_p.tile([116, GW], bf16, tag="cre")
                cim = sp_p.tile([116, GW], bf16, tag="cim")
                for g in range(40):
                    rg = g + (1 if g >= 32 else 0)
                    nc.vector.scalar_tensor_tensor(
                        out=cre[:, 256 * g:256 * g + 256], in0=whre,
                        scalar=mT2[:, rg:rg + 1], in1=wre,
                        op0=Alu.mult, op1=Alu.add)
                    nc.vector.scalar_tensor_tensor(
                        out=cim[:, 256 * g:256 * g + 256], in0=whim,
                        scalar=mT2[:, rg:rg + 1], in1=wim,
                        op0=Alu.mult, op1=Alu.add)
                # in-place products: P = X*CRE, Q = X*CIM
                nc.vector.tensor_tensor(out=cre, in0=xb, in1=cre, op=Alu.mult)
                nc.vector.tensor_tensor(out=cim, in0=xb, in1=cim, op=Alu.mult)

                st0 = st_p.tile([127, FW], bf16, tag="st0")
                st1 = st_p.tile([114, FW], bf16, tag="st1")
                for g in range(40):
                    sps = saps_p.tile([102, 256], f32, tag="sa")
                    nc.tensor.matmul(sps, ainv_sb[:, 102 * g:102 * g + 102],
                                     cre[:, 256 * g:256 * g + 256],
                                     start=True, stop=False)
                    nc.tensor.matmul(sps, ainvs_sb[:, 102 * g:102 * g + 102],
                                     cim[:, 256 * g:256 * g + 256],
                                     start=False, stop=True)
                    ssb = s_p.tile([102, 256], bf16, tag="s")
                    nc.scalar.copy(out=ssb, in_=sps)
                    ctdst = (st0[6 * g + 1:6 * g + 7, :] if g <= 20 else
                             st1[6 * (g - 21):6 * (g - 21) + 6, :]).rearrange(
                                 "i (q c) -> i q c", q=17, c=256)
                    cteng2 = (nc.gpsimd, nc.scalar)[g % 2]
                    cteng2.dma_start(out=ctdst, in_=ssb)
                # class 0
                cre0 = y_p.tile([42, 256], bf16, tag="cre0")
                cim0 = y_p.tile([42, 256], bf16, tag="cim0")
                nc.vector.scalar_tensor_tensor(
                    out=cre0, in0=whre[0:42, :], scalar=m0c, in1=wre[0:42, :],
                    op0=Alu.mult, op1=Alu.add)
                nc.vector.scalar_tensor_tensor(
                    out=cim0, in0=whim[0:42, :], scalar=m0c, in1=wim[0:42, :],
                    op0=Alu.mult, op1=Alu.add)
                nc.vector.tensor_tensor(out=cre0, in0=x0f, in1=cre0, op=Alu.mult)
                nc.vector.tensor_tensor(out=cim0, in0=x0f, in1=cim0, op=Alu.mult)
                s0ps = saps_p.tile([18, 256], f32, tag="sa")
                nc.tensor.matmul(s0ps, ainv0_sb, cre0, start=True, stop=False)
                nc.tensor.matmul(s0ps, ainv0s_sb, cim0, start=False, stop=True)
                s0sb = s_p.tile([18, 256], bf16, tag="sc0")
                nc.scalar.copy(out=s0sb, in_=s0ps)
                nc.sync.dma_start(
                    out=st0[0:1, :].rearrange("i (q c) -> i q c", q=17, c=256),
                    in_=s0sb[0:17, :])

                ov = out_t.ap().rearrange("s (a b) c -> s a b c", a=241, b=17)
                for mt in range(2):
                    for fc in range(NCH):
                        lo, w = chunk(fc)
                        ps = sbps_p.tile([128, 512], f32, tag="sb")
                        nc.tensor.matmul(ps[:, 0:w],
                                         b1k0[:, 128 * mt:128 * mt + 128],
                                         st0[:, lo:lo + w],
                                         start=True, stop=False)
                        nc.tensor.matmul(ps[:, 0:w],
                                         b1k1[:, 128 * mt:128 * mt + 128],
                                         st1[:, lo:lo + w],
                                         start=False, stop=True)
                        rows = 128 if mt == 0 else 113
                        osb = o_p.tile([128, 512], f32, tag="osb")
                        if fc % 2 == 0:
                            nc.vector.tensor_copy(out=osb[0:rows, 0:w],
                                                  in_=ps[0:rows, 0:w])
                        else:
                            nc.scalar.copy(out=osb[0:rows, 0:w],
                                           in_=ps[0:rows, 0:w])
                        n2lo, n2n = lo // 256, (w + 255) // 256
                        nc.sync.dma_start(
                            out=ov[s:s + 1, 128 * mt:128 * mt + rows,
                                   n2lo:n2lo + n2n, :],
                            in_=osb[0:rows, 0:w].rearrange(
                                "p (q c) -> p q c", q=n2n, c=256))

    nc.compile()
    _NC_CACHE["nc"] = nc
    return nc


def kernel(x_in, complex_weight, complex_weight_high, threshold_param):
    from concourse.bass_utils import run_bass_kernel_spmd
    nc = _build_nc()
    bf = ml_dtypes.bfloat16

    thrp = np.asarray(threshold_param, np.float32)[_BINM.reshape(-1)]
    thrp = np.ascontiguousarray(thrp.reshape(42, 51))
    cw = np.asarray(complex_weight, np.float32)
    cwh = np.asarray(complex_weight_high, np.float32)
    wre = np.ascontiguousarray(np.broadcast_to(cw[:, 0], (116, C))).astype(bf)
    wim = np.ascontiguousarray(np.broadcast_to(cw[:, 1], (116, C))).astype(bf)
    whre = np.ascontiguousarray(np.broadcast_to(cwh[:, 0], (116, C))).astype(bf)
    whim = np.ascontiguousarray(np.broadcast_to(cwh[:, 1], (116, C))).astype(bf)

    x_in = np.ascontiguousarray(np.asarray(x_in, np.float32))
    in_maps = []
    for core in range(8):
        m = {"x": x_in[BL * core:BL * core + BL],
             "thrp": thrp, "wre": wre, "wim": wim,
             "whre": whre, "whim": whim}
        m.update(_CONSTS)
        in_maps.append(m)
    res = run_bass_kernel_spmd(nc, in_maps, core_ids=list(range(8)))
    out = np.concatenate([res.results[i]["out"] for i in range(8)], axis=0)
    return out.astype(np.float32)
